# revision 1
# baseline (speedup 1.0000x reference)
"""NystromAttention kernel: data-parallel over batch across 8 NeuronCores.

Strategy (per sharding_hint): batch dim 32 -> 8 shards of 4; to_qkv/to_out
weights replicated per core. The Moore-Penrose pinv init scale in the
reference is a GLOBAL max over all (b, h) landmark matrices, so it is
precomputed on host (cheap: landmark pooling commutes with the linear
projection) and broadcast to every shard — keeping per-shard results
bit-compatible with the unsharded reference up to fp32 rounding.
"""

import numpy as np

HEADS = 8
DIM_HEAD = 64
DIM = 512
NUM_LANDMARKS = 256
PINV_ITERS = 6
KS = 33
N_CORES = 8

B, C, H, W = 32, 512, 32, 32
N = H * W                 # 1024 tokens
L = N // NUM_LANDMARKS    # 4 tokens per landmark


def _softmax_np(s):
    s = s - s.max(axis=-1, keepdims=True)
    e = np.exp(s)
    return e / e.sum(axis=-1, keepdims=True)


def _global_pinv_scale(x, w_qkv):
    """Reference's jnp.max(col) * jnp.max(row) over the FULL batch.

    Landmark mean pooling commutes with the qkv projection, so attn2 can be
    reproduced from pooled tokens without the full n x n work.
    """
    b = x.shape[0]
    h, d, m = HEADS, DIM_HEAD, NUM_LANDMARKS
    seq = np.ascontiguousarray(x.transpose(0, 2, 3, 1)).reshape(b, N, C)
    seq_land = seq.reshape(b, m, L, C).mean(axis=2)          # [b, m, C]
    flat = seq_land.reshape(b * m, C)
    q_land = (flat @ w_qkv[:, : h * d]).reshape(b, m, h, d)
    k_land = (flat @ w_qkv[:, h * d : 2 * h * d]).reshape(b, m, h, d)
    q_land = q_land.transpose(0, 2, 1, 3) * (d ** -0.5)      # [b, h, m, d]
    k_land = k_land.transpose(0, 2, 1, 3)
    sim2 = np.einsum("bhid,bhjd->bhij", q_land, k_land)
    attn2 = _softmax_np(sim2)
    ax = np.abs(attn2)
    col = ax.sum(axis=-1)
    row = ax.sum(axis=-2)
    return np.float32(col.max() * row.max())


def _shard_fn_factory(jnp):
    d = DIM_HEAD
    h, m = HEADS, NUM_LANDMARKS

    def pinv(x, inv_scale):
        z = jnp.swapaxes(x, -1, -2) * inv_scale
        I = jnp.eye(x.shape[-1], dtype=x.dtype)
        for _ in range(PINV_ITERS):
            xz = x @ z
            z = 0.25 * z @ (13.0 * I - xz @ (15.0 * I - xz @ (7.0 * I - xz)))
        return z

    def shard_fn(x, w_qkv, w_out, b_out, res_kernel, inv_scale):
        b = x.shape[0]
        seq = x.transpose(0, 2, 3, 1).reshape(b, N, C)
        qkv = seq @ w_qkv
        q, k, v = jnp.split(qkv, 3, axis=-1)
        to_heads = lambda t: t.reshape(b, N, h, d).transpose(0, 2, 1, 3)
        q, k, v = to_heads(q), to_heads(k), to_heads(v)
        q = q * (d ** -0.5)

        q_land = q.reshape(b, h, m, L, d).mean(axis=3)
        k_land = k.reshape(b, h, m, L, d).mean(axis=3)

        sim1 = jnp.einsum("bhid,bhjd->bhij", q, k_land)
        sim2 = jnp.einsum("bhid,bhjd->bhij", q_land, k_land)
        sim3 = jnp.einsum("bhid,bhjd->bhij", q_land, k)

        import jax

        attn1 = jax.nn.softmax(sim1, axis=-1)
        attn2 = jax.nn.softmax(sim2, axis=-1)
        attn3 = jax.nn.softmax(sim3, axis=-1)

        attn2_inv = pinv(attn2, inv_scale)
        out = (attn1 @ attn2_inv) @ (attn3 @ v)

        # depthwise conv over sequence dim as 33 shifted MACs (avoids
        # lax.conv on the experimental backend)
        pad = KS // 2
        vp = jnp.pad(v, ((0, 0), (0, 0), (pad, pad), (0, 0)))
        wk = res_kernel[:, 0, :, 0]  # [h, KS]
        res = jnp.zeros_like(v)
        for kk in range(KS):
            res = res + wk[None, :, kk, None, None] * vp[:, :, kk : kk + N, :]
        out = out + res

        out = out.transpose(0, 2, 1, 3).reshape(b, N, h * d)
        out = out @ w_out + b_out
        return out.reshape(b, H, W, C).transpose(0, 3, 1, 2)

    return shard_fn


def _run_jax(x, w_qkv, w_out, b_out, res_kernel, inv_scale):
    import jax
    import jax.numpy as jnp

    devs = jax.devices()[:N_CORES]
    if len(devs) < N_CORES:
        raise RuntimeError("not enough devices")
    shard_fn = _shard_fn_factory(jnp)
    pm = jax.pmap(
        shard_fn,
        in_axes=(0, None, None, None, None, None),
        devices=devs,
    )
    xs = x.reshape(N_CORES, B // N_CORES, C, H, W)
    out = pm(xs, w_qkv, w_out, b_out, res_kernel, inv_scale)
    out = np.asarray(out, dtype=np.float32).reshape(B, C, H, W)
    if not np.isfinite(out).all():
        raise RuntimeError("non-finite output from device path")
    return out


def _run_numpy(x, w_qkv, w_out, b_out, res_kernel):
    b = x.shape[0]
    h, d, m = HEADS, DIM_HEAD, NUM_LANDMARKS
    seq = np.ascontiguousarray(x.transpose(0, 2, 3, 1)).reshape(b, N, C)
    qkv = seq.reshape(b * N, C) @ w_qkv
    qkv = qkv.reshape(b, N, 3 * h * d)
    q, k, v = np.split(qkv, 3, axis=-1)
    to_heads = lambda t: np.ascontiguousarray(
        t.reshape(b, N, h, d).transpose(0, 2, 1, 3)
    )
    q, k, v = to_heads(q), to_heads(k), to_heads(v)
    q = q * (d ** -0.5)

    q_land = q.reshape(b, h, m, L, d).mean(axis=3)
    k_land = k.reshape(b, h, m, L, d).mean(axis=3)

    sim1 = np.einsum("bhid,bhjd->bhij", q, k_land, optimize=True)
    sim2 = np.einsum("bhid,bhjd->bhij", q_land, k_land, optimize=True)
    sim3 = np.einsum("bhid,bhjd->bhij", q_land, k, optimize=True)
    attn1 = _softmax_np(sim1)
    attn2 = _softmax_np(sim2)
    attn3 = _softmax_np(sim3)

    ax = np.abs(attn2)
    z = np.swapaxes(attn2, -1, -2) / (ax.sum(-1).max() * ax.sum(-2).max())
    I = np.eye(m, dtype=attn2.dtype)
    for _ in range(PINV_ITERS):
        xz = attn2 @ z
        z = 0.25 * z @ (13.0 * I - xz @ (15.0 * I - xz @ (7.0 * I - xz)))

    out = (attn1 @ z) @ (attn3 @ v)

    pad = KS // 2
    vp = np.pad(v, ((0, 0), (0, 0), (pad, pad), (0, 0)))
    wk = res_kernel[:, 0, :, 0]
    res = np.zeros_like(v)
    for kk in range(KS):
        res += wk[None, :, kk, None, None] * vp[:, :, kk : kk + N, :]
    out = out + res

    out = out.transpose(0, 2, 1, 3).reshape(b, N, h * d)
    out = out @ w_out + b_out
    return np.ascontiguousarray(
        out.reshape(b, H, W, C).transpose(0, 3, 1, 2)
    ).astype(np.float32)


def kernel(x, w_qkv, w_out, b_out, res_kernel):
    x = np.asarray(x, dtype=np.float32)
    w_qkv = np.asarray(w_qkv, dtype=np.float32)
    w_out = np.asarray(w_out, dtype=np.float32)
    b_out = np.asarray(b_out, dtype=np.float32)
    res_kernel = np.asarray(res_kernel, dtype=np.float32)

    scale = _global_pinv_scale(x, w_qkv)
    inv_scale = np.float32(1.0) / scale
    try:
        return _run_jax(x, w_qkv, w_out, b_out, res_kernel, inv_scale)
    except Exception:
        return _run_numpy(x, w_qkv, w_out, b_out, res_kernel)


# revision 2
# speedup vs baseline: 2.2721x; 2.2721x over previous
"""NystromAttention kernel: data-parallel over batch across 8 NeuronCores.

Strategy (per sharding_hint): batch dim 32 -> 8 shards of 4 per core;
to_qkv/to_out weights replicated. The landmark matrices and pinv iterations
stay local per shard. The Moore-Penrose pinv init scale in the reference is
a GLOBAL max over all (b, h) landmark matrices, so shards compute local
col/row-sum maxes and combine them with an 8-way lax.pmax (scalar
collective) — matching the unsharded reference up to fp32 rounding.

Fallbacks: pmax collective unsupported -> host-precomputed global scale;
device path fails entirely -> numpy.
"""

import numpy as np

HEADS = 8
DIM_HEAD = 64
DIM = 512
NUM_LANDMARKS = 256
PINV_ITERS = 6
KS = 33
N_CORES = 8

B, C, H, W = 32, 512, 32, 32
N = H * W                 # 1024 tokens
L = N // NUM_LANDMARKS    # 4 tokens per landmark

_PMAP_CACHE = {}


def _softmax_np(s):
    s = s - s.max(axis=-1, keepdims=True)
    e = np.exp(s)
    return e / e.sum(axis=-1, keepdims=True)


def _global_pinv_scale(x, w_qkv):
    """Reference's jnp.max(col) * jnp.max(row) over the FULL batch (host).

    Landmark mean pooling commutes with the qkv projection, so attn2 is
    reproduced from pooled tokens without the full n x n work.
    """
    b = x.shape[0]
    h, d, m = HEADS, DIM_HEAD, NUM_LANDMARKS
    seq = np.ascontiguousarray(x.transpose(0, 2, 3, 1)).reshape(b, N, C)
    seq_land = seq.reshape(b, m, L, C).mean(axis=2)          # [b, m, C]
    flat = seq_land.reshape(b * m, C)
    q_land = (flat @ w_qkv[:, : h * d]).reshape(b, m, h, d)
    k_land = (flat @ w_qkv[:, h * d : 2 * h * d]).reshape(b, m, h, d)
    q_land = np.ascontiguousarray(q_land.transpose(0, 2, 1, 3)) * (d ** -0.5)
    k_land = np.ascontiguousarray(k_land.transpose(0, 2, 1, 3))
    sim2 = np.matmul(q_land, np.swapaxes(k_land, -1, -2))
    attn2 = _softmax_np(sim2)
    ax = np.abs(attn2)
    return np.float32(ax.sum(axis=-1).max() * ax.sum(axis=-2).max())


def _shard_fn_factory(jnp, use_pmax):
    d = DIM_HEAD
    h, m = HEADS, NUM_LANDMARKS

    def shard_fn(x, w_qkv, w_out, b_out, res_kernel, inv_scale):
        import jax

        b = x.shape[0]
        seq = x.transpose(0, 2, 3, 1).reshape(b, N, C)
        qkv = seq @ w_qkv
        q, k, v = jnp.split(qkv, 3, axis=-1)
        to_heads = lambda t: t.reshape(b, N, h, d).transpose(0, 2, 1, 3)
        q, k, v = to_heads(q), to_heads(k), to_heads(v)
        q = q * (d ** -0.5)

        q_land = q.reshape(b, h, m, L, d).mean(axis=3)
        k_land = k.reshape(b, h, m, L, d).mean(axis=3)

        sim1 = jnp.einsum("bhid,bhjd->bhij", q, k_land)
        sim2 = jnp.einsum("bhid,bhjd->bhij", q_land, k_land)
        sim3 = jnp.einsum("bhid,bhjd->bhij", q_land, k)

        attn1 = jax.nn.softmax(sim1, axis=-1)
        attn2 = jax.nn.softmax(sim2, axis=-1)
        attn3 = jax.nn.softmax(sim3, axis=-1)

        # Moore-Penrose pinv, 6 Newton-Schulz iterations
        ax = jnp.abs(attn2)
        if use_pmax:
            col_max = jax.lax.pmax(ax.sum(axis=-1).max(), axis_name="cores")
            row_max = jax.lax.pmax(ax.sum(axis=-2).max(), axis_name="cores")
            z = jnp.swapaxes(attn2, -1, -2) / (col_max * row_max)
        else:
            z = jnp.swapaxes(attn2, -1, -2) * inv_scale
        I = jnp.eye(m, dtype=attn2.dtype)
        for _ in range(PINV_ITERS):
            xz = attn2 @ z
            z = 0.25 * z @ (13.0 * I - xz @ (15.0 * I - xz @ (7.0 * I - xz)))

        out = (attn1 @ z) @ (attn3 @ v)

        # depthwise conv over sequence dim as 33 shifted MACs (avoids
        # lax.conv on the experimental backend)
        pad = KS // 2
        vp = jnp.pad(v, ((0, 0), (0, 0), (pad, pad), (0, 0)))
        wk = res_kernel[:, 0, :, 0]  # [h, KS]
        res = jnp.zeros_like(v)
        for kk in range(KS):
            res = res + wk[None, :, kk, None, None] * vp[:, :, kk : kk + N, :]
        out = out + res

        out = out.transpose(0, 2, 1, 3).reshape(b, N, h * d)
        out = out @ w_out + b_out
        return out.reshape(b, H, W, C).transpose(0, 3, 1, 2)

    return shard_fn


def _get_pmap(use_pmax):
    key = ("pmap", use_pmax)
    if key not in _PMAP_CACHE:
        import jax
        import jax.numpy as jnp

        devs = jax.devices()[:N_CORES]
        if len(devs) < N_CORES:
            raise RuntimeError("not enough devices")
        shard_fn = _shard_fn_factory(jnp, use_pmax)
        _PMAP_CACHE[key] = jax.pmap(
            shard_fn,
            axis_name="cores",
            in_axes=(0, None, None, None, None, None),
            devices=devs,
        )
    return _PMAP_CACHE[key]


def _run_jax(x, w_qkv, w_out, b_out, res_kernel, use_pmax):
    inv_scale = (
        np.float32(0.0)
        if use_pmax
        else np.float32(1.0) / _global_pinv_scale(x, w_qkv)
    )
    pm = _get_pmap(use_pmax)
    xs = x.reshape(N_CORES, B // N_CORES, C, H, W)
    out = pm(xs, w_qkv, w_out, b_out, res_kernel, inv_scale)
    out = np.asarray(out, dtype=np.float32).reshape(B, C, H, W)
    if not np.isfinite(out).all():
        raise RuntimeError("non-finite output from device path")
    return out


def _run_numpy(x, w_qkv, w_out, b_out, res_kernel):
    b = x.shape[0]
    h, d, m = HEADS, DIM_HEAD, NUM_LANDMARKS
    seq = np.ascontiguousarray(x.transpose(0, 2, 3, 1)).reshape(b, N, C)
    qkv = (seq.reshape(b * N, C) @ w_qkv).reshape(b, N, 3 * h * d)
    q, k, v = np.split(qkv, 3, axis=-1)
    to_heads = lambda t: np.ascontiguousarray(
        t.reshape(b, N, h, d).transpose(0, 2, 1, 3)
    )
    q, k, v = to_heads(q), to_heads(k), to_heads(v)
    q = q * (d ** -0.5)

    q_land = q.reshape(b, h, m, L, d).mean(axis=3)
    k_land = k.reshape(b, h, m, L, d).mean(axis=3)

    sim1 = np.matmul(q, np.swapaxes(k_land, -1, -2))
    sim2 = np.matmul(q_land, np.swapaxes(k_land, -1, -2))
    sim3 = np.matmul(q_land, np.swapaxes(k, -1, -2))
    attn1 = _softmax_np(sim1)
    attn2 = _softmax_np(sim2)
    attn3 = _softmax_np(sim3)

    ax = np.abs(attn2)
    z = np.swapaxes(attn2, -1, -2) / (ax.sum(-1).max() * ax.sum(-2).max())
    I = np.eye(m, dtype=attn2.dtype)
    for _ in range(PINV_ITERS):
        xz = attn2 @ z
        z = 0.25 * z @ (13.0 * I - xz @ (15.0 * I - xz @ (7.0 * I - xz)))

    out = (attn1 @ z) @ (attn3 @ v)

    pad = KS // 2
    vp = np.pad(v, ((0, 0), (0, 0), (pad, pad), (0, 0)))
    wk = res_kernel[:, 0, :, 0]
    res = np.zeros_like(v)
    for kk in range(KS):
        res += wk[None, :, kk, None, None] * vp[:, :, kk : kk + N, :]
    out = out + res

    out = out.transpose(0, 2, 1, 3).reshape(b, N, h * d)
    out = out @ w_out + b_out
    return np.ascontiguousarray(
        out.reshape(b, H, W, C).transpose(0, 3, 1, 2)
    ).astype(np.float32)


def kernel(x, w_qkv, w_out, b_out, res_kernel):
    x = np.asarray(x, dtype=np.float32)
    w_qkv = np.asarray(w_qkv, dtype=np.float32)
    w_out = np.asarray(w_out, dtype=np.float32)
    b_out = np.asarray(b_out, dtype=np.float32)
    res_kernel = np.asarray(res_kernel, dtype=np.float32)

    for use_pmax in (True, False):
        try:
            return _run_jax(x, w_qkv, w_out, b_out, res_kernel, use_pmax)
        except Exception:
            continue
    return _run_numpy(x, w_qkv, w_out, b_out, res_kernel)
